# revision 55
# baseline (speedup 1.0000x reference)
"""GAT layer kernel for Trainium2, 8-core data-parallel over batch.

Math (per batch b, head h):
    h = x @ W                              [N, H*HD]
    s_i = <h_i, a_src[h]>,  t_j = <h_j, a_dst[h]>
    A[j, i] = exp(leakyrelu(s_i + t_j, 0.2))
    out[i]  = (sum_j A[j, i] * h_j) / (sum_j A[j, i])

Softmax over j is invariant to any per-column (per-i) scaling, so scale
column i by e^{-s_i}:
    Ā[j, i] = e^{-s_i} A[j, i] = max(e^{t_j}, e^{0.2 t_j} * e^{-0.8 s_i})
(exact: for s+t>=0 the left branch wins and equals e^{s+t-s}; below, the
right branch.) The left branch has no i-dependence, so each [128, N]
attention tile is ONE DVE tensor_scalar op with two per-partition scalars:
    a = (es08_bcast * etc02_col) max etc_col
in bf16 (2x DVE rate). The only broadcast tensor is es08[h] = e^{-0.8 s},
one per head, built by a rank-1-weight PE matmul (stride-0 AP view of
W @ a_src[h] replicated over 128 weight columns) into one-bank PSUM
chunks, exp'd to SBUF by ACT per chunk — no DRAM round-trips.

t is produced directly in column form (t[j] on partitions) by
matmul(xT_tile, W @ a_dst) per node tile — no transpose hop.

All PE inputs are bf16: PE streams bf16 at 1 col/cycle but f16 at only
1/2 col/cycle (measured 0.42 vs 0.84 ns/col), and bf16 input rounding
(~0.4%) matches the bf16 A-tile noise floor. The whole input arrives as
one contiguous [Wa | W | xT] bf16 block on the gpsimd software DGE
(which spreads descriptors over all 16 DMA engines; the HWDGE queues
serialize ~90ns/descriptor on one engine).

Aggregation: out^T[(h,d)|Z, i] accumulated in PSUM with a [h_node | 1s]
weight block (Z row = column sums of Ā). Per head, the raw [33, N]
accumulator (numerator + Z row) is copied to SBUF by the otherwise idle
ACT engine and DMA'd out raw; the division by Z, transpose to node-major
and f32 cast happen on the host during unsharding. The last head's copy
and DMA are split into column halves to shorten the trailing chain.
"""

import numpy as np

B, N, IN_F, OUT_F, H = 8, 1024, 128, 128, 4
HD = OUT_F // H  # 32
NEG = 0.2
N_CORES = 8
NT = N // 128  # 8 node tiles

_CACHE = {}


def _build_nc():
    import concourse.bacc as bacc
    import concourse.tile as tile
    from concourse import mybir

    f32 = mybir.dt.float32
    bf16 = mybir.dt.bfloat16
    AF = mybir.ActivationFunctionType
    ALU = mybir.AluOpType

    nc = bacc.Bacc("TRN2", target_bir_lowering=False, debug=False,
                   num_devices=N_CORES)

    # contiguous input block: [Wa (2H) | W (128) | xT (1024)]
    CW = 2 * H + OUT_F + N
    inp = nc.declare_dram_parameter("inp", [IN_F, CW], bf16, isOutput=False)
    # onum[r, h*N + i]: row r<32 = numerator dim, row 32 = Z; heads 0-2
    # leave in one wide DMA (single descriptor-gen), head 3 in its own.
    # bf16 halves the output bytes; Z in bf16 costs ~0.4% on the quotient.
    onum = nc.declare_dram_parameter("onum", [33, H * N], bf16, isOutput=True)

    with tile.TileContext(nc) as tc:
      with (
        tc.tile_pool(name="const", bufs=1) as cpool,
        tc.tile_pool(name="atile", bufs=12) as apool,
        tc.tile_pool(name="otile", bufs=2) as opool,
      ):
        # ---- load input in two partition halves on the gpsimd SWDGE ----
        inp_sb = cpool.tile([IN_F, CW], bf16, tag="inp")
        nc.gpsimd.dma_start(out=inp_sb[0:64, :], in_=inp[0:64, :])
        nc.gpsimd.dma_start(out=inp_sb[64:IN_F, :], in_=inp[64:IN_F, :])
        Wa_sb = inp_sb[:, 0:2 * H]
        W_sb = inp_sb[:, 2 * H:2 * H + OUT_F]
        xT_sb = inp_sb[:, 2 * H + OUT_F:CW]

        # prime the ACT exp table before it's on the critical path
        warm = cpool.tile([1, 8], f32, tag="warm")
        nc.scalar.activation(out=warm, in_=inp_sb[0:1, 0:8], func=AF.Exp)

        # materialized rank-1 weight tiles (ACT stride-0 copy): a real SBUF
        # weight streams at full PE rate, the stride-0 AP does not. Head 0
        # first — it gates the first attention tile; the rest are emitted
        # lazily between the es08 exps.
        wsbt = {}

        def emit_wsbt(h):
            wt_h = cpool.tile([IN_F, 128], bf16, tag=f"wsbt{h}")
            nc.scalar.copy(out=wt_h,
                           in_=Wa_sb[:, h:h + 1].to_broadcast([IN_F, 128]))
            wsbt[h] = wt_h

        emit_wsbt(0)

        es08_b = {}
        for h in range(H):
            eb = cpool.tile([128, N], bf16, tag=f"es08b{h}")
            es08_b[h] = eb

        # ---- t columns first (they unlock the whole DVE pipeline):
        # tc_ps[j, 4*jt + h] = t_h[128*jt + j], own 1-bank pool
        pstc = tc.tile_pool(name="ps_tc", bufs=1, space="PSUM")
        tcpool = pstc.__enter__()
        tc_ps = tcpool.tile([128, H * NT], f32, tag="tc")
        for jt in range(NT):
            nc.tensor.matmul(tc_ps[:, H * jt:H * (jt + 1)],
                             xT_sb[:, 128 * jt:128 * (jt + 1)],
                             Wa_sb[:, H:2 * H], start=True, stop=True)
        etc = cpool.tile([128, H * NT], f32, tag="etc")
        nc.scalar.activation(out=etc, in_=tc_ps, func=AF.Exp)
        etc02 = cpool.tile([128, H * NT], f32, tag="etc02")
        nc.scalar.activation(out=etc02, in_=tc_ps, func=AF.Exp, scale=NEG)

        # ---- one-bank [128, 512] PSUM ring shared by the es08 broadcast
        # chunks and the hn (h_node) tiles, interleaved into the main loop
        ps512 = tc.tile_pool(name="ps512", bufs=3, space="PSUM")
        ring = ps512.__enter__()

        def emit_sbcast_chunk(h, c):
            sb_ps = ring.tile([128, 512], f32, tag="ps512")
            nc.tensor.matmul(sb_ps, wsbt[h],
                             xT_sb[:, 512 * c:512 * (c + 1)],
                             start=True, stop=True)
            nc.scalar.activation(out=es08_b[h][:, 512 * c:512 * (c + 1)],
                                 in_=sb_ps, func=AF.Exp, scale=-0.8)

        # weight tiles: wt[:, 132jt+33h : +32] = h_node, col 32 = 1s
        wt_all = cpool.tile([128, NT * 33 * H], bf16, tag="wt")
        wt_v = wt_all[:].rearrange("p (jt h c) -> p jt h c", h=H, c=33)
        nc.vector.memset(wt_v[:, :, :, 32:33], 1.0)
        wts = [wt_all[:, 132 * jt:132 * (jt + 1)] for jt in range(NT)]

        def emit_hn_batch(g):
            # 2 node-tiles per ring slot, one wt copy per batch — small
            # batches so the first wt block lands early
            hn_t = ring.tile([128, 512], f32, tag="ps512")
            for q in range(2):
                jt = 2 * g + q
                nc.tensor.matmul(hn_t[:, 128 * q:128 * (q + 1)],
                                 xT_sb[:, 128 * jt:128 * (jt + 1)], W_sb,
                                 start=True, stop=True)
            nc.vector.tensor_copy(
                out=wt_v[:, 2 * g:2 * (g + 1), :, 0:32],
                in_=hn_t[:, 0:256].rearrange("p (jt h c) -> p jt h c",
                                             h=H, c=32))

        # numerator + Z rows: idle ACT engine copies PSUM into one wide
        # staging tile; heads 0-2 ship as ONE DMA (one descriptor-gen on
        # the serialized gpsimd queue), head 3 ships alone, split into
        # column halves to shorten the trailing chain.
        ocp_all = cpool.tile([33, H * N], bf16, tag="ocp")

        def emit_out(h, oh, split=False):
            base = N * h
            if split:
                # last head: DVE takes one column half (it is idle by now),
                # ACT the other, each half shipped as soon as it lands
                nc.vector.tensor_copy(out=ocp_all[:, base:base + 512],
                                      in_=oh[:, 0:512])
                nc.gpsimd.dma_start(out=onum[:, base:base + 512],
                                    in_=ocp_all[:, base:base + 512])
                nc.scalar.copy(out=ocp_all[:, base + 512:base + N],
                               in_=oh[:, 512:N])
                nc.gpsimd.dma_start(out=onum[:, base + 512:base + N],
                                    in_=ocp_all[:, base + 512:base + N])
            else:
                nc.scalar.copy(out=ocp_all[:, base:base + N], in_=oh)
            if h == H - 2:
                nc.gpsimd.dma_start(out=onum[:, 0:3 * N],
                                    in_=ocp_all[:, 0:3 * N])

        # all es08 broadcasts and hn tiles are emitted up-front: the a_t
        # stream (DVE) is the loop's critical path, so every es08 exp must
        # land before its head's a_t run starts; the scheduler slots the
        # main matmuls into the PE gaps.
        emit_sbcast_chunk(0, 0)
        emit_sbcast_chunk(0, 1)
        with tc.tile_pool(name="ps_main", bufs=2, space="PSUM") as psmain:
            emit_hn_batch(0)
            emit_hn_batch(1)
            for h in range(1, H):
                emit_wsbt(h)
                emit_sbcast_chunk(h, 0)
                emit_sbcast_chunk(h, 1)
            emit_hn_batch(2)
            emit_hn_batch(3)
            ohs = [None] * H
            for h in range(H):
                oh = psmain.tile([33, N], f32, tag="oh")
                ohs[h] = oh
                for jt in range(NT):
                    idx = H * jt + h
                    a_t = apool.tile([128, N], bf16, tag="at")
                    if h == 0 and jt == 0:
                        # split the very first tile so the first main matmul
                        # chunk starts half an exp earlier
                        for c in range(2):
                            nc.vector.tensor_scalar(
                                out=a_t[:, 512 * c:512 * (c + 1)],
                                in0=es08_b[h][:, 512 * c:512 * (c + 1)],
                                scalar1=etc02[:, idx:idx + 1],
                                scalar2=etc[:, idx:idx + 1],
                                op0=ALU.mult, op1=ALU.max)
                    else:
                        nc.vector.tensor_scalar(
                            out=a_t, in0=es08_b[h],
                            scalar1=etc02[:, idx:idx + 1],
                            scalar2=etc[:, idx:idx + 1],
                            op0=ALU.mult, op1=ALU.max)
                    for c in range(2):
                        nc.tensor.matmul(
                            oh[:, 512 * c:512 * (c + 1)],
                            wts[jt][:, 33 * h:33 * (h + 1)],
                            a_t[:, 512 * c:512 * (c + 1)],
                            start=(jt == 0), stop=(jt == NT - 1))
                if h >= 1:
                    emit_out(h - 1, ohs[h - 1])
            emit_out(H - 1, ohs[H - 1], split=True)
        ps512.__exit__(None, None, None)
        pstc.__exit__(None, None, None)

    nc.compile()
    return nc


def _get_nc():
    if "nc" not in _CACHE:
        _CACHE["nc"] = _build_nc()
    return _CACHE["nc"]


def _prep_in_maps(x, W, a_src, a_dst):
    import ml_dtypes
    bf = ml_dtypes.bfloat16

    x = np.asarray(x, dtype=np.float32)
    W = np.asarray(W, dtype=np.float32)
    a_src = np.asarray(a_src, dtype=np.float32)
    a_dst = np.asarray(a_dst, dtype=np.float32)

    a_ext = np.zeros((OUT_F, 2 * H), np.float32)
    for h in range(H):
        a_ext[h * HD:(h + 1) * HD, h] = a_src[h]
        a_ext[h * HD:(h + 1) * HD, H + h] = a_dst[h]
    Wa = (W @ a_ext).astype(bf)
    W16 = W.astype(bf)

    return [
        {"inp": np.ascontiguousarray(np.concatenate(
            [Wa, W16, x[c].T.astype(bf)], axis=1))}
        for c in range(N_CORES)
    ]


def kernel(x, W, a_src, a_dst):
    from concourse.bass_utils import run_bass_kernel_spmd

    nc = _get_nc()
    in_maps = _prep_in_maps(x, W, a_src, a_dst)
    res = run_bass_kernel_spmd(nc, in_maps, core_ids=list(range(N_CORES)))
    out = np.empty((N_CORES, N, OUT_F), np.float32)
    for c in range(N_CORES):
        o = res.results[c]["onum"].astype(np.float32).reshape(33, H, N)
        out[c] = (o[0:HD, :, :] / o[HD:HD + 1, :, :]).transpose(2, 1, 0) \
            .reshape(N, OUT_F)
    return np.ascontiguousarray(out)


# revision 56
# speedup vs baseline: 1.0264x; 1.0264x over previous
"""GAT layer kernel for Trainium2, 8-core data-parallel over batch.

Math (per batch b, head h):
    h = x @ W                              [N, H*HD]
    s_i = <h_i, a_src[h]>,  t_j = <h_j, a_dst[h]>
    A[j, i] = exp(leakyrelu(s_i + t_j, 0.2))
    out[i]  = (sum_j A[j, i] * h_j) / (sum_j A[j, i])

Softmax over j is invariant to any per-column (per-i) scaling, so scale
column i by e^{-s_i}:
    Ā[j, i] = e^{-s_i} A[j, i] = max(e^{t_j}, e^{0.2 t_j} * e^{-0.8 s_i})
(exact: for s+t>=0 the left branch wins and equals e^{s+t-s}; below, the
right branch.) The left branch has no i-dependence, so each [128, N]
attention tile is ONE DVE tensor_scalar op with two per-partition scalars:
    a = (es08_bcast * etc02_col) max etc_col
in bf16 (2x DVE rate). The only broadcast tensor is es08[h] = e^{-0.8 s},
one per head, built by a rank-1-weight PE matmul (weight tile = column
W @ a_src[h] replicated 128x, materialized by an ACT stride-0 copy so
the PE streams it at full rate) into one-bank PSUM chunks, exp'd to
SBUF by ACT per chunk — no DRAM round-trips.

t is produced directly in column form (t[j] on partitions) by
matmul(xT_tile, W @ a_dst) per node tile — no transpose hop.

All PE inputs are bf16: PE streams bf16 at 1 col/cycle but f16 at only
1/2 col/cycle (measured 0.42 vs 0.84 ns/col), and bf16 input rounding
(~0.4%) matches the bf16 A-tile noise floor. The whole input arrives as
one contiguous [Wa | W | xT] bf16 block on the gpsimd software DGE
(which spreads descriptors over all 16 DMA engines; the HWDGE queues
serialize ~90ns/descriptor on one engine).

Aggregation: out^T[(h,d)|Z, i] accumulated in PSUM with a [h_node | 1s]
weight block (Z row = column sums of Ā). Per head, the raw [33, N]
accumulator (numerator + Z row) is copied to SBUF by the otherwise idle
ACT engine and DMA'd out raw; the division by Z, transpose to node-major
and f32 cast happen on the host during unsharding. The last head's copy
and DMA are split into column halves to shorten the trailing chain.
"""

import numpy as np

B, N, IN_F, OUT_F, H = 8, 1024, 128, 128, 4
HD = OUT_F // H  # 32
NEG = 0.2
N_CORES = 8
NT = N // 128  # 8 node tiles

_CACHE = {}


def _build_nc():
    import concourse.bacc as bacc
    import concourse.tile as tile
    from concourse import mybir

    f32 = mybir.dt.float32
    bf16 = mybir.dt.bfloat16
    AF = mybir.ActivationFunctionType
    ALU = mybir.AluOpType

    nc = bacc.Bacc("TRN2", target_bir_lowering=False, debug=False,
                   num_devices=N_CORES)

    # contiguous input block: [Wa (2H) | W (128) | xT (1024)]
    CW = 2 * H + OUT_F + N
    inp = nc.declare_dram_parameter("inp", [IN_F, CW], bf16, isOutput=False)
    # onum[r, h*N + i]: row r<32 = numerator dim, row 32 = Z; heads 0-2
    # leave in one wide DMA (single descriptor-gen), head 3 in its own.
    # bf16 halves the output bytes; Z in bf16 costs ~0.4% on the quotient.
    onum = nc.declare_dram_parameter("onum", [33, H * N], bf16, isOutput=True)

    with tile.TileContext(nc) as tc:
      with (
        tc.tile_pool(name="const", bufs=1) as cpool,
        tc.tile_pool(name="atile", bufs=12) as apool,
        tc.tile_pool(name="otile", bufs=2) as opool,
      ):
        # ---- load input in two partition halves on the gpsimd SWDGE ----
        inp_sb = cpool.tile([IN_F, CW], bf16, tag="inp")
        nc.gpsimd.dma_start(out=inp_sb[0:64, :], in_=inp[0:64, :])
        nc.gpsimd.dma_start(out=inp_sb[64:IN_F, :], in_=inp[64:IN_F, :])
        Wa_sb = inp_sb[:, 0:2 * H]
        W_sb = inp_sb[:, 2 * H:2 * H + OUT_F]
        xT_sb = inp_sb[:, 2 * H + OUT_F:CW]

        # prime the ACT exp table before it's on the critical path
        warm = cpool.tile([1, 8], f32, tag="warm")
        nc.scalar.activation(out=warm, in_=inp_sb[0:1, 0:8], func=AF.Exp)

        # materialized rank-1 weight tiles (ACT stride-0 copy): a real SBUF
        # weight streams at full PE rate, the stride-0 AP does not. Head 0
        # first — it gates the first attention tile; the rest are emitted
        # lazily between the es08 exps.
        wsbt = {}

        def emit_wsbt(h):
            wt_h = cpool.tile([IN_F, 128], bf16, tag=f"wsbt{h}")
            nc.scalar.copy(out=wt_h,
                           in_=Wa_sb[:, h:h + 1].to_broadcast([IN_F, 128]))
            wsbt[h] = wt_h

        emit_wsbt(0)

        es08_b = {}
        for h in range(H):
            eb = cpool.tile([128, N], bf16, tag=f"es08b{h}")
            es08_b[h] = eb

        # ---- t columns first (they unlock the whole DVE pipeline):
        # tc_ps[j, 4*jt + h] = t_h[128*jt + j], own 1-bank pool
        pstc = tc.tile_pool(name="ps_tc", bufs=1, space="PSUM")
        tcpool = pstc.__enter__()
        tc_ps = tcpool.tile([128, H * NT], f32, tag="tc")
        for jt in range(NT):
            nc.tensor.matmul(tc_ps[:, H * jt:H * (jt + 1)],
                             xT_sb[:, 128 * jt:128 * (jt + 1)],
                             Wa_sb[:, H:2 * H], start=True, stop=True)
        etc = cpool.tile([128, H * NT], f32, tag="etc")
        nc.scalar.activation(out=etc, in_=tc_ps, func=AF.Exp)
        etc02 = cpool.tile([128, H * NT], f32, tag="etc02")
        nc.scalar.activation(out=etc02, in_=tc_ps, func=AF.Exp, scale=NEG)

        # ---- one-bank [128, 512] PSUM ring shared by the es08 broadcast
        # chunks and the hn (h_node) tiles, interleaved into the main loop
        ps512 = tc.tile_pool(name="ps512", bufs=3, space="PSUM")
        ring = ps512.__enter__()

        def emit_sbcast_chunk(h, c):
            sb_ps = ring.tile([128, 512], f32, tag="ps512")
            nc.tensor.matmul(sb_ps, wsbt[h],
                             xT_sb[:, 512 * c:512 * (c + 1)],
                             start=True, stop=True)
            nc.scalar.activation(out=es08_b[h][:, 512 * c:512 * (c + 1)],
                                 in_=sb_ps, func=AF.Exp, scale=-0.8)

        # weight tiles: wt[:, 132jt+33h : +32] = h_node, col 32 = 1s
        wt_all = cpool.tile([128, NT * 33 * H], bf16, tag="wt")
        wt_v = wt_all[:].rearrange("p (jt h c) -> p jt h c", h=H, c=33)
        nc.vector.memset(wt_v[:, :, :, 32:33], 1.0)
        wts = [wt_all[:, 132 * jt:132 * (jt + 1)] for jt in range(NT)]

        def emit_hn_batch(g):
            # 2 node-tiles per ring slot, one wt copy per batch — small
            # batches so the first wt block lands early
            hn_t = ring.tile([128, 512], f32, tag="ps512")
            for q in range(2):
                jt = 2 * g + q
                nc.tensor.matmul(hn_t[:, 128 * q:128 * (q + 1)],
                                 xT_sb[:, 128 * jt:128 * (jt + 1)], W_sb,
                                 start=True, stop=True)
            nc.vector.tensor_copy(
                out=wt_v[:, 2 * g:2 * (g + 1), :, 0:32],
                in_=hn_t[:, 0:256].rearrange("p (jt h c) -> p jt h c",
                                             h=H, c=32))

        # numerator + Z rows: idle ACT engine copies PSUM into one wide
        # staging tile; heads 0-2 ship as ONE DMA (one descriptor-gen on
        # the serialized gpsimd queue), head 3 ships alone, split into
        # column halves to shorten the trailing chain.
        ocp_all = cpool.tile([33, H * N], bf16, tag="ocp")

        def emit_out(h, oh, split=False):
            base = N * h
            if split:
                # last head: DVE takes one column half (it is idle by now),
                # ACT the other, each half shipped as soon as it lands
                nc.vector.tensor_copy(out=ocp_all[:, base:base + 512],
                                      in_=oh[:, 0:512])
                nc.gpsimd.dma_start(out=onum[:, base:base + 512],
                                    in_=ocp_all[:, base:base + 512])
                nc.scalar.copy(out=ocp_all[:, base + 512:base + N],
                               in_=oh[:, 512:N])
                nc.gpsimd.dma_start(out=onum[:, base + 512:base + N],
                                    in_=ocp_all[:, base + 512:base + N])
            else:
                nc.scalar.copy(out=ocp_all[:, base:base + N], in_=oh)
            if h == H - 2:
                nc.gpsimd.dma_start(out=onum[:, 0:3 * N],
                                    in_=ocp_all[:, 0:3 * N])

        # all es08 broadcasts and hn tiles are emitted up-front: the a_t
        # stream (DVE) is the loop's critical path, so every es08 exp must
        # land before its head's a_t run starts; the scheduler slots the
        # main matmuls into the PE gaps.
        emit_sbcast_chunk(0, 0)
        emit_sbcast_chunk(0, 1)
        with tc.tile_pool(name="ps_main", bufs=2, space="PSUM") as psmain:
            emit_hn_batch(0)
            emit_hn_batch(1)
            for h in range(1, H):
                emit_wsbt(h)
                emit_sbcast_chunk(h, 0)
                emit_sbcast_chunk(h, 1)
            emit_hn_batch(2)
            emit_hn_batch(3)
            ohs = [None] * H
            for h in range(H):
                oh = psmain.tile([33, N], f32, tag="oh")
                ohs[h] = oh
                for jt in range(NT):
                    idx = H * jt + h
                    a_t = apool.tile([128, N], bf16, tag="at")
                    if h == 0 and jt == 0:
                        # split the very first tile so the first main matmul
                        # chunk starts half an exp earlier
                        for c in range(2):
                            nc.vector.tensor_scalar(
                                out=a_t[:, 512 * c:512 * (c + 1)],
                                in0=es08_b[h][:, 512 * c:512 * (c + 1)],
                                scalar1=etc02[:, idx:idx + 1],
                                scalar2=etc[:, idx:idx + 1],
                                op0=ALU.mult, op1=ALU.max)
                    else:
                        nc.vector.tensor_scalar(
                            out=a_t, in0=es08_b[h],
                            scalar1=etc02[:, idx:idx + 1],
                            scalar2=etc[:, idx:idx + 1],
                            op0=ALU.mult, op1=ALU.max)
                    for c in range(2):
                        nc.tensor.matmul(
                            oh[:, 512 * c:512 * (c + 1)],
                            wts[jt][:, 33 * h:33 * (h + 1)],
                            a_t[:, 512 * c:512 * (c + 1)],
                            start=(jt == 0), stop=(jt == NT - 1))
                if h >= 1:
                    emit_out(h - 1, ohs[h - 1])
            emit_out(H - 1, ohs[H - 1], split=True)
        ps512.__exit__(None, None, None)
        pstc.__exit__(None, None, None)

    nc.compile()
    return nc


def _get_nc():
    if "nc" not in _CACHE:
        _CACHE["nc"] = _build_nc()
    return _CACHE["nc"]


def _prep_in_maps(x, W, a_src, a_dst):
    import ml_dtypes
    bf = ml_dtypes.bfloat16

    x = np.asarray(x, dtype=np.float32)
    W = np.asarray(W, dtype=np.float32)
    a_src = np.asarray(a_src, dtype=np.float32)
    a_dst = np.asarray(a_dst, dtype=np.float32)

    a_ext = np.zeros((OUT_F, 2 * H), np.float32)
    for h in range(H):
        a_ext[h * HD:(h + 1) * HD, h] = a_src[h]
        a_ext[h * HD:(h + 1) * HD, H + h] = a_dst[h]
    Wa = (W @ a_ext).astype(bf)
    W16 = W.astype(bf)

    return [
        {"inp": np.ascontiguousarray(np.concatenate(
            [Wa, W16, x[c].T.astype(bf)], axis=1))}
        for c in range(N_CORES)
    ]


def kernel(x, W, a_src, a_dst):
    from concourse.bass_utils import run_bass_kernel_spmd

    nc = _get_nc()
    in_maps = _prep_in_maps(x, W, a_src, a_dst)
    res = run_bass_kernel_spmd(nc, in_maps, core_ids=list(range(N_CORES)))
    out = np.empty((N_CORES, N, OUT_F), np.float32)
    for c in range(N_CORES):
        o = res.results[c]["onum"].astype(np.float32).reshape(33, H, N)
        out[c] = (o[0:HD, :, :] / o[HD:HD + 1, :, :]).transpose(2, 1, 0) \
            .reshape(N, OUT_F)
    return np.ascontiguousarray(out)
